# revision 33
# baseline (speedup 1.0000x reference)
"""Trainium2 Bass kernel for causal multi-head attention (B=4, T=2048, C=1024, H=16).

Sharding: tensor-parallel over heads x batch. 8 cores = 4 batches x 2 head-halves.
Each core computes, for its batch b and its 8 heads (4 head-pairs):
  qkv projection -> causal attention -> output projection partial (rows of w_proj)
Host gathers by summing the two half-partials per batch (the "all-reduce").

v2 pipeline design (vs the phase-serial v1):
  - Everything bf16 (x, w_attn, w_proj): measured end-to-end rel err ~5e-3 vs
    the 2e-2 gate; halves input DMA + SBUF and enables fast weight load.
  - Single scheduling scope: projection t-chunks, attention q-chunks and the
    output projection are emitted interleaved (proj c0, attn q0, proj c1,
    oproj q0, attn q1, ...) so the Tile list-scheduler overlaps the ACT-bound
    softmax with PE-bound projection work instead of serializing phases.
  - Scores for a head PAIR run as two concurrent row-tiled matmuls
    (tile_size 64x128, tile_position (0,0)/(64,0)): contraction is d=64 so
    two heads share the 128x128 PE array -> 2x on the QK^T stream.
  - Scores land in one [128, 2{head}, 512] PSUM tile (two banks); exp for both
    heads is a single ACT instruction; the diagonal-block causal mask is one
    DVE multiply over both heads.
  - Softmax denominator: ones-column folded into V (row 64 of PV output);
    evacuated from PSUM by ACT (func=Copy shares the exp table), reciprocal'd
    on DVE after a DRAM-bounce partition-broadcast.
PSUM budget: proj/oproj accum 2 banks + scores 2x2 banks + PV pair 2 banks = 8.
"""

import sys

for _p in ("/opt/trn_rl_repo",):
    if _p not in sys.path:
        sys.path.insert(0, _p)

import numpy as np

import concourse.bass as bass
import concourse.mybir as mybir
import concourse.tile as tile
from concourse import bacc
from concourse.bass import ts
from concourse.bass_utils import run_bass_kernel_spmd

B, T, C, H, D = 4, 2048, 1024, 16, 64
NCORES = 8
JC = 512  # channels per core (8 heads x 64)
HL = 8  # heads per core
NP = 4  # head pairs per core
CT = C // 128  # 8 contraction tiles
NCH = T // 512  # 4 t/q chunks
F32 = mybir.dt.float32
BF16 = mybir.dt.bfloat16
EXP = mybir.ActivationFunctionType.Exp
CPY = mybir.ActivationFunctionType.Copy
ADD = mybir.AluOpType.add
MULT = mybir.AluOpType.mult


def _trace(nc, tc, io):
    xT, wq, wk, wv, wp, bq, bk, bv, bp, tri2, out = io

    with (
        tc.tile_pool(name="consts", bufs=1) as consts,
        tc.tile_pool(name="wts", bufs=1) as w_pool,
        tc.tile_pool(name="qk", bufs=1) as qk_pool,
        tc.tile_pool(name="vp", bufs=1) as v_pool,
        tc.tile_pool(name="yp", bufs=1) as y_pool,
        tc.tile_pool(name="xt", bufs=2) as xt_pool,
        tc.tile_pool(name="pp", bufs=2, space="PSUM") as ppsum,
        tc.tile_pool(name="sc", bufs=2, space="PSUM") as sc_pool,
        tc.tile_pool(name="pv", bufs=1, space="PSUM") as pv_pool,
        tc.tile_pool(name="pt", bufs=4) as pt_pool,
        tc.tile_pool(name="pvs", bufs=3) as pvs_pool,
        tc.tile_pool(name="rd", bufs=3) as rd_pool,
        tc.tile_pool(name="dsc", bufs=4, space="DRAM") as d_pool,
        tc.tile_pool(name="ob", bufs=2) as o_pool,
        tc.tile_pool(name="op3", bufs=8) as part_pool,
    ):
        # ---- weights + consts. All large inputs are host-pre-swizzled to
        # partition-major layout so every DMA has 8KB-contiguous lines per
        # partition (1KB lines measured ~2.5x slower). wq and x chunk 0
        # first: they gate the first projection matmuls. -----------------
        # HWDGE DMAs are FIFO per issuing engine; split the startup loads
        # across BOTH rings (sync + scalar) so wq/xt0 stream in parallel.
        # Piece-split the critical first loads (2 ct per piece = 2KB lines):
        # SDMA engines round-robin ALL pending queues, so (a) completion
        # granularity decides when the first matmuls start and (b) later
        # loads must be HELD BACK or they steal startup bandwidth. Tiny
        # SBUF->SBUF probe DMAs create real completion dependencies:
        # wk/wv dispatch only after wq/xt0 have fully landed.
        wq_sb = w_pool.tile([128, CT, JC], BF16, tag="wq")
        xt0 = xt_pool.tile([128, CT, 512], BF16, tag="xt", name="xt0")
        for lo, hi in ((0, 1), (1, 2), (2, 4), (4, 8)):
            nc.sync.dma_start(out=wq_sb[:, lo:hi, :], in_=wq[:, lo:hi, :])
            nc.scalar.dma_start(out=xt0[:, lo:hi, :], in_=xT[0][:, lo:hi, :])
        wk_sb = w_pool.tile([128, CT, JC], BF16, tag="wk")
        wv_sb = w_pool.tile([128, CT, JC], BF16, tag="wv")
        nc.sync.dma_start(out=wk_sb[0:1, :, 0:1], in_=wq_sb[0:1, :, 0:1])
        nc.scalar.dma_start(out=wv_sb[0:1, :, 0:1], in_=xt0[0:1, :, 0:1])
        for c2 in range(0, CT, 2):
            nc.sync.dma_start(out=wk_sb[:, c2 : c2 + 2, :], in_=wk[:, c2 : c2 + 2, :])
            nc.scalar.dma_start(out=wv_sb[:, c2 : c2 + 2, :], in_=wv[:, c2 : c2 + 2, :])
        xt1 = xt_pool.tile([128, CT, 512], BF16, tag="xt", name="xt1")
        nc.sync.dma_start(out=xt1[0:1, 0:1, 0:1], in_=wk_sb[0:1, 0:1, 0:1])
        nc.sync.dma_start(out=xt1, in_=xT[1])
        tri_sb = consts.tile([128, 2, 128], BF16, tag="tri")
        nc.scalar.dma_start(out=tri_sb, in_=tri2)
        bq_sb = consts.tile([128, 4], F32, tag="bq")
        nc.scalar.dma_start(out=bq_sb, in_=bq.rearrange("(jt p) -> p jt", p=128))
        bk_sb = consts.tile([128, 4], F32, tag="bk")
        nc.scalar.dma_start(out=bk_sb, in_=bk.rearrange("(jt p) -> p jt", p=128))
        bv_sb = consts.tile([128, JC], F32, tag="bv")
        nc.scalar.dma_start(out=bv_sb, in_=bv.unsqueeze(0).to_broadcast([128, JC]))
        bp_sb = consts.tile([128, C], F32, tag="bp")
        wp_sb = w_pool.tile([128, 4, C], BF16, tag="wp")

        def load_wp():
            # deferred: wp/bp are first needed by oproj q0, well after
            # startup; the scheduler places these after attn q0's exps in
            # the scalar ring so they don't steal startup DMA bandwidth.
            nc.scalar.dma_start(out=wp_sb, in_=wp)
            nc.scalar.dma_start(out=bp_sb, in_=bp.unsqueeze(0).to_broadcast([128, C]))

        q_sb = [
            qk_pool.tile([128, T], BF16, tag=f"q{jt}", name=f"q{jt}")
            for jt in range(NP)
        ]
        k_sb = [
            qk_pool.tile([128, T], BF16, tag=f"k{jt}", name=f"k{jt}")
            for jt in range(NP)
        ]
        v_sb = [
            v_pool.tile([128, HL, 65], BF16, tag=f"v{tt}", name=f"v{tt}")
            for tt in range(T // 128)
        ]
        y_sb = [
            y_pool.tile([128, T], BF16, tag=f"y{jt}", name=f"y{jt}")
            for jt in range(NP)
        ]

        xts = {0: xt0, 1: xt1}

        def proj_units(c):
            # generator: yields after each psum-group (12 units per chunk)
            # prefetch chunk c+1 (its pool slot frees when chunk c-1 retires)
            if c + 1 <= 3 and c + 1 not in xts:
                nxt = xt_pool.tile([128, CT, 512], BF16, tag="xt", name=f"xt{c + 1}")
                nc.sync.dma_start(out=nxt, in_=xT[c + 1])
                xts[c + 1] = nxt
            xt_t = xts[c]
            for wsb, bsb, dst in ((wq_sb, bq_sb, q_sb), (wk_sb, bk_sb, k_sb)):
                for jt in range(NP):
                    ps = ppsum.tile([128, 512], F32, tag="pp")
                    for ct in range(CT):
                        nc.tensor.matmul(
                            ps,
                            lhsT=wsb[:, ct, ts(jt, 128)],
                            rhs=xt_t[:, ct, :],
                            start=(ct == 0),
                            stop=(ct == CT - 1),
                        )
                    nc.vector.tensor_scalar_add(
                        out=dst[jt][:, ts(c, 512)], in0=ps, scalar1=bsb[:, jt : jt + 1]
                    )
                    yield
            for sub in range(4):
                t128 = 4 * c + sub
                ps = ppsum.tile([128, 512], F32, tag="pp")
                for ct in range(CT):
                    nc.tensor.matmul(
                        ps,
                        lhsT=xt_t[:, ct, ts(sub, 128)],
                        rhs=wv_sb[:, ct, :],
                        start=(ct == 0),
                        stop=(ct == CT - 1),
                    )
                nc.vector.memset(v_sb[t128][:, :, 64:65], 1.0)
                nc.vector.tensor_tensor(
                    out=v_sb[t128][:, :, 0:64],
                    in0=ps.rearrange("p (h d) -> p h d", h=HL),
                    in1=bv_sb.rearrange("p (h d) -> p h d", h=HL),
                    op=ADD,
                )
                yield

        def attn_units(qc, prs=None):
            for pr in prs if prs is not None else range(NP):
                hA, hB = 2 * pr, 2 * pr + 1
                # one PV accumulation group per head, each owning a full PSUM
                # bank (start=True clears has_written for the WHOLE bank, so
                # two groups must never share one).
                pv = pv_pool.tile([128, 2, 512], F32, tag="pv", name=f"pv{qc}_{pr}")
                lastkt = 4 * qc + 3
                for kt in range(lastkt + 1):
                    off = max(0, 128 * (kt - 4 * qc))
                    sc = sc_pool.tile(
                        [128, 2, 512], F32, tag="sc", name=f"sc{qc}_{pr}_{kt}"
                    )
                    nc.tensor.matmul(
                        sc[:, 0, off:512],
                        lhsT=k_sb[pr][0:64, ts(kt, 128)],
                        rhs=q_sb[pr][0:64, 512 * qc + off : 512 * (qc + 1)],
                        start=True,
                        stop=True,
                        tile_position=(0, 0),
                    )
                    nc.tensor.matmul(
                        sc[:, 1, off:512],
                        lhsT=k_sb[pr][64:128, ts(kt, 128)],
                        rhs=q_sb[pr][64:128, 512 * qc + off : 512 * (qc + 1)],
                        start=True,
                        stop=True,
                        tile_position=(64, 0),
                    )
                    pt = pt_pool.tile(
                        [128, 2, 512], BF16, tag="pt", name=f"pt{qc}_{pr}_{kt}"
                    )
                    nc.scalar.activation(
                        out=pt[:, :, off:512], in_=sc[:, :, off:512], func=EXP, scale=0.125
                    )
                    if kt >= 4 * qc:
                        # causal mask of the diagonal 128x128 block, both heads
                        nc.vector.tensor_tensor(
                            out=pt[:, :, off : off + 128],
                            in0=pt[:, :, off : off + 128],
                            in1=tri_sb,
                            op=MULT,
                        )
                    for i, h in ((0, hA), (1, hB)):
                        nc.tensor.matmul(
                            pv[0:65, i, off:512],
                            lhsT=v_sb[kt][:, h, :],
                            rhs=pt[:, i, off:512],
                            start=(kt == 0),
                            stop=(kt == lastkt),
                        )
                    if kt != lastkt:
                        yield
                # evacuate PV psum in one DVE copy so the banks free fast;
                # the denominator DMA-bounce chain then runs off SBUF,
                # off the Tensor engine's critical path.
                pvs = pvs_pool.tile([65, 2, 512], F32, tag="pvs", name=f"pvs{qc}_{pr}")
                nc.vector.tensor_copy(out=pvs, in_=pv[0:65, :, :])
                dscr = d_pool.tile([1024], F32, tag="dscr", name=f"ds{qc}_{pr}")
                nc.sync.dma_start(out=dscr.unsqueeze(0), in_=pvs[64:65, :, :])
                rdb = rd_pool.tile([64, 1024], F32, tag="rdb", name=f"rb{qc}_{pr}")
                nc.sync.dma_start(
                    out=rdb, in_=dscr.unsqueeze(0).to_broadcast([64, 1024])
                )
                rcp = rd_pool.tile([64, 1024], F32, tag="rcp", name=f"rc{qc}_{pr}")
                nc.vector.reciprocal_approx_fast(out=rcp, in_=rdb)
                for i in range(2):
                    nc.vector.tensor_tensor(
                        out=y_sb[pr][64 * i : 64 * i + 64, ts(qc, 512)],
                        in0=pvs[0:64, i, :],
                        in1=rcp[:, ts(i, 512)],
                        op=MULT,
                    )
                yield

        def oproj_units(qc):
            for sub in range(4):
                t128 = 4 * qc + sub
                ot = o_pool.tile([128, C], F32, tag="ot", name=f"ot{t128}")
                for ch in range(2):
                    ps = ppsum.tile([128, 512], F32, tag="pp", name=f"op{t128}_{ch}")
                    for jt in range(NP):
                        nc.tensor.matmul(
                            ps,
                            lhsT=y_sb[jt][:, ts(t128, 128)],
                            rhs=wp_sb[:, jt, ts(ch, 512)],
                            start=(jt == 0),
                            stop=(jt == NP - 1),
                        )
                    nc.vector.tensor_tensor(
                        out=ot[:, ts(ch, 512)], in0=ps, in1=bp_sb[:, ts(ch, 512)], op=ADD
                    )
                nc.sync.dma_start(out=out[ts(t128, 128), :], in_=ot)
                yield

        op3_parts = {}

        def oproj3_pre():
            # jt 0-2 partial output projection for the last q chunk: runs
            # during the (ACT-bound) final attention pair, evacuated to SBUF
            # so only the 8 jt3 matmuls + adds remain after the last norm.
            for sub in range(4):
                t128 = 12 + sub
                for ch in range(2):
                    ps = ppsum.tile([128, 512], F32, tag="pp", name=f"o3a{sub}_{ch}")
                    for jt in range(3):
                        nc.tensor.matmul(
                            ps,
                            lhsT=y_sb[jt][:, ts(t128, 128)],
                            rhs=wp_sb[:, jt, ts(ch, 512)],
                            start=(jt == 0),
                            stop=(jt == 2),
                        )
                    part = part_pool.tile(
                        [128, 512], F32, tag="part", name=f"o3p{sub}_{ch}"
                    )
                    nc.vector.tensor_tensor(
                        out=part, in0=ps, in1=bp_sb[:, ts(ch, 512)], op=ADD
                    )
                    op3_parts[(sub, ch)] = part
                    yield

        def oproj3_post():
            for sub in range(4):
                t128 = 12 + sub
                ot = o_pool.tile([128, C], F32, tag="ot", name=f"ot{t128}")
                for ch in range(2):
                    ps = ppsum.tile([128, 512], F32, tag="pp", name=f"o3b{sub}_{ch}")
                    nc.tensor.matmul(
                        ps,
                        lhsT=y_sb[3][:, ts(t128, 128)],
                        rhs=wp_sb[:, 3, ts(ch, 512)],
                        start=True,
                        stop=True,
                    )
                    nc.vector.tensor_tensor(
                        out=ot[:, ts(ch, 512)],
                        in0=ps,
                        in1=op3_parts[(sub, ch)],
                        op=ADD,
                    )
                nc.sync.dma_start(out=out[ts(t128, 128), :], in_=ot)

        def chain(*gens):
            for g in gens:
                yield from g

        def drive(primary, filler, ratio):
            # emit filler units between primary units at `ratio` fillers per
            # primary: puts guaranteed-ready PE work right behind each
            # exp-gated attention unit in every engine queue.
            acc = 0.0
            filler_alive = True
            for _ in primary:
                acc += ratio
                while acc >= 1.0 and filler_alive:
                    try:
                        next(filler)
                    except StopIteration:
                        filler_alive = False
                    acc -= 1.0
            if filler_alive:
                for _ in filler:
                    pass

        # staircase: attn q-chunk c trails proj chunk c; oproj trails attn.
        # Attention units lead (their scores feed the ACT-bound softmax);
        # projection/oproj groups are interleaved as ready PE filler.
        for _ in proj_units(0):
            pass
        load_wp()
        drive(attn_units(0), proj_units(1), 12.0 / 16)
        drive(attn_units(1), chain(proj_units(2), oproj_units(0)), 16.0 / 32)
        drive(attn_units(2), chain(proj_units(3), oproj_units(1)), 16.0 / 48)
        drive(attn_units(3, range(3)), oproj_units(2), 4.0 / 48)
        drive(attn_units(3, (3,)), oproj3_pre(), 6.0 / 16)
        oproj3_post()


_CACHE = {}


def build_nc():
    if "nc" in _CACHE:
        return _CACHE["nc"]
    nc = bacc.Bacc(
        "TRN2",
        target_bir_lowering=False,
        debug=False,
        enable_asserts=False,
        num_devices=NCORES,
    )
    io = (
        nc.dram_tensor("xT", [NCH, 128, CT, 512], BF16, kind="ExternalInput").ap(),
        nc.dram_tensor("wq", [128, CT, JC], BF16, kind="ExternalInput").ap(),
        nc.dram_tensor("wk", [128, CT, JC], BF16, kind="ExternalInput").ap(),
        nc.dram_tensor("wv", [128, CT, JC], BF16, kind="ExternalInput").ap(),
        nc.dram_tensor("wp", [128, 4, C], BF16, kind="ExternalInput").ap(),
        nc.dram_tensor("bq", [JC], F32, kind="ExternalInput").ap(),
        nc.dram_tensor("bk", [JC], F32, kind="ExternalInput").ap(),
        nc.dram_tensor("bv", [JC], F32, kind="ExternalInput").ap(),
        nc.dram_tensor("bp", [C], F32, kind="ExternalInput").ap(),
        nc.dram_tensor("tri", [128, 2, 128], BF16, kind="ExternalInput").ap(),
        nc.dram_tensor("out", [T, C], F32, kind="ExternalOutput").ap(),
    )
    with tile.TileContext(nc) as tc:
        _trace(nc, tc, io)
    nc.compile()
    _CACHE["nc"] = nc
    return nc


def make_in_maps(x, w_attn, b_attn, w_proj, b_proj):
    import ml_dtypes

    BF = ml_dtypes.bfloat16

    def _swz_w(w):  # [C, JC] -> [128p, CT, JC], partition-major contiguous
        return np.ascontiguousarray(
            w.reshape(CT, 128, JC).transpose(1, 0, 2)
        ).astype(BF)

    tri = np.triu(np.ones((128, 128), dtype=np.float32))
    tri2 = np.ascontiguousarray(np.stack([tri, tri], axis=1)).astype(BF)
    zeros_c = np.zeros(C, dtype=np.float32)
    in_maps = []
    for core in range(NCORES):
        b, hh = core // 2, core % 2
        j0 = JC * hh
        # x[b].T is [C, T]; -> [chunk, 128p, ct, 512t] contiguous
        xTs = np.ascontiguousarray(
            x[b].T.reshape(CT, 128, NCH, 512).transpose(2, 1, 0, 3)
        ).astype(BF)
        in_maps.append(
            {
                "xT": xTs,
                "wq": _swz_w(w_attn[:, j0 : j0 + JC]),
                "wk": _swz_w(w_attn[:, C + j0 : C + j0 + JC]),
                "wv": _swz_w(w_attn[:, 2 * C + j0 : 2 * C + j0 + JC]),
                "wp": np.ascontiguousarray(
                    w_proj[j0 : j0 + JC, :].reshape(4, 128, C).transpose(1, 0, 2)
                ).astype(BF),
                "bq": np.ascontiguousarray(b_attn[j0 : j0 + JC]).astype(np.float32),
                "bk": np.ascontiguousarray(b_attn[C + j0 : C + j0 + JC]).astype(
                    np.float32
                ),
                "bv": np.ascontiguousarray(b_attn[2 * C + j0 : 2 * C + j0 + JC]).astype(
                    np.float32
                ),
                "bp": (b_proj.astype(np.float32) if hh == 0 else zeros_c),
                "tri": tri2,
            }
        )
    return in_maps


def gather(parts):
    out = np.empty((B, T, C), dtype=np.float32)
    for b in range(B):
        out[b] = parts[2 * b]["out"] + parts[2 * b + 1]["out"]
    return out


def kernel(x, w_attn, b_attn, w_proj, b_proj):
    x = np.asarray(x, dtype=np.float32)
    w_attn = np.asarray(w_attn, dtype=np.float32)
    b_attn = np.asarray(b_attn, dtype=np.float32)
    w_proj = np.asarray(w_proj, dtype=np.float32)
    b_proj = np.asarray(b_proj, dtype=np.float32)
    nc = build_nc()
    in_maps = make_in_maps(x, w_attn, b_attn, w_proj, b_proj)
    res = run_bass_kernel_spmd(nc, in_maps, core_ids=list(range(NCORES)))
    return gather(res.results)


if __name__ == "__main__":
    rng = np.random.default_rng(0)
    x = rng.standard_normal((B, T, C), dtype=np.float32)
    w_attn = rng.standard_normal((C, 3 * C), dtype=np.float32) / np.sqrt(C)
    b_attn = np.zeros(3 * C, np.float32)
    w_proj = rng.standard_normal((C, C), dtype=np.float32) / np.sqrt(C)
    b_proj = np.zeros(C, np.float32)
    out = kernel(x, w_attn, b_attn, w_proj, b_proj)
    print(out.shape, out.dtype, np.abs(out).mean())


# revision 35
# speedup vs baseline: 1.0121x; 1.0121x over previous
"""Trainium2 Bass kernel for causal multi-head attention (B=4, T=2048, C=1024, H=16).

Sharding: tensor-parallel over heads x batch. 8 cores = 4 batches x 2 head-halves.
Each core computes, for its batch b and its 8 heads (4 head-pairs):
  qkv projection -> causal attention -> output projection partial (rows of w_proj)
Host gathers by summing the two half-partials per batch (the "all-reduce").

Pipeline design (vs the phase-serial v1 at ~452us; this version ~289us):
  - Everything bf16 (x, w_attn, w_proj): measured end-to-end rel err 5.4e-3 vs
    the 2e-2 gate; halves input DMA + SBUF and enables fast weight load.
  - Single scheduling scope, generator-driven emission: attention (pair, kt)
    units lead and projection / output-projection psum-groups are interleaved
    between them at unit granularity ("drive"), so every engine queue has
    guaranteed-ready PE work right behind each exp-gated attention unit. The
    staircase proj c0 -> attn q0 || proj c1 -> ... overlaps the ACT-bound
    softmax with PE-bound projection instead of serializing phases.
  - Scores for a head PAIR run as two concurrent row-tiled matmuls
    (tile_size 64x128, tile_position (0,0)/(64,0)): contraction is d=64 so
    two heads share the 128x128 PE array -> 2x on the QK^T stream. Both
    heads land in one [128, 2, 512] PSUM tile (separate banks, required by
    row tiling); exp for both heads is a single ACT instruction; the
    diagonal-block causal mask is one DVE multiply over both heads.
  - Softmax denominator: ones-column folded into V (row 64 of the PV psum).
    One accumulation group per head, each owning a full PSUM bank (start=True
    clears has_written for the WHOLE bank - groups must never share one).
    PV psum is evacuated in one DVE copy (frees banks fast; keeps the
    DMA-bounce reciprocal-broadcast chain off the Tensor critical path).
  - Startup: HWDGE DMAs are FIFO per issuing engine and SDMA engines
    round-robin all pending queues, so the first loads are piece-split
    (progressive completion) across BOTH rings (sync + scalar) and later
    loads are held back by tiny SBUF->SBUF probe DMAs (real completion
    deps); wp/bp are deferred past the attention-q0 exps.
  - Tail: the last q-chunk's output projection is split jt0-2 (pre-run
    during the ACT-bound final attention pair, evacuated to SBUF partials)
    + jt3-only short pass after the final normalize.
PSUM budget: proj/oproj accum 2 banks + scores 2x2 banks + PV pair 2 banks = 8.
"""

import sys

for _p in ("/opt/trn_rl_repo",):
    if _p not in sys.path:
        sys.path.insert(0, _p)

import numpy as np

import concourse.bass as bass
import concourse.mybir as mybir
import concourse.tile as tile
from concourse import bacc
from concourse.bass import ts
from concourse.bass_utils import run_bass_kernel_spmd

B, T, C, H, D = 4, 2048, 1024, 16, 64
NCORES = 8
JC = 512  # channels per core (8 heads x 64)
HL = 8  # heads per core
NP = 4  # head pairs per core
CT = C // 128  # 8 contraction tiles
NCH = T // 512  # 4 t/q chunks
F32 = mybir.dt.float32
BF16 = mybir.dt.bfloat16
EXP = mybir.ActivationFunctionType.Exp
CPY = mybir.ActivationFunctionType.Copy
ADD = mybir.AluOpType.add
MULT = mybir.AluOpType.mult


def _trace(nc, tc, io):
    xT, wq, wk, wv, wp, bq, bk, bv, bp, tri2, out = io

    with (
        tc.tile_pool(name="consts", bufs=1) as consts,
        tc.tile_pool(name="wts", bufs=1) as w_pool,
        tc.tile_pool(name="qk", bufs=1) as qk_pool,
        tc.tile_pool(name="vp", bufs=1) as v_pool,
        tc.tile_pool(name="yp", bufs=1) as y_pool,
        tc.tile_pool(name="xt", bufs=2) as xt_pool,
        tc.tile_pool(name="pp", bufs=2, space="PSUM") as ppsum,
        tc.tile_pool(name="sc", bufs=2, space="PSUM") as sc_pool,
        tc.tile_pool(name="pv", bufs=1, space="PSUM") as pv_pool,
        tc.tile_pool(name="pt", bufs=4) as pt_pool,
        tc.tile_pool(name="pvs", bufs=3) as pvs_pool,
        tc.tile_pool(name="rd", bufs=3) as rd_pool,
        tc.tile_pool(name="dsc", bufs=4, space="DRAM") as d_pool,
        tc.tile_pool(name="ob", bufs=2) as o_pool,
        tc.tile_pool(name="op3", bufs=8) as part_pool,
    ):
        # ---- weights + consts. All large inputs are host-pre-swizzled to
        # partition-major layout so every DMA has 8KB-contiguous lines per
        # partition (1KB lines measured ~2.5x slower). wq and x chunk 0
        # first: they gate the first projection matmuls. -----------------
        # HWDGE DMAs are FIFO per issuing engine; split the startup loads
        # across BOTH rings (sync + scalar) so wq/xt0 stream in parallel.
        # Piece-split the critical first loads (2 ct per piece = 2KB lines):
        # SDMA engines round-robin ALL pending queues, so (a) completion
        # granularity decides when the first matmuls start and (b) later
        # loads must be HELD BACK or they steal startup bandwidth. Tiny
        # SBUF->SBUF probe DMAs create real completion dependencies:
        # wk/wv dispatch only after wq/xt0 have fully landed.
        wq_sb = w_pool.tile([128, CT, JC], BF16, tag="wq")
        xt0 = xt_pool.tile([128, CT, 512], BF16, tag="xt", name="xt0")
        for lo, hi in ((0, 1), (1, 2), (2, 4), (4, 8)):
            nc.sync.dma_start(out=wq_sb[:, lo:hi, :], in_=wq[:, lo:hi, :])
            nc.scalar.dma_start(out=xt0[:, lo:hi, :], in_=xT[0][:, lo:hi, :])
        wk_sb = w_pool.tile([128, CT, JC], BF16, tag="wk")
        wv_sb = w_pool.tile([128, CT, JC], BF16, tag="wv")
        nc.sync.dma_start(out=wk_sb[0:1, :, 0:1], in_=wq_sb[0:1, :, 0:1])
        nc.scalar.dma_start(out=wv_sb[0:1, :, 0:1], in_=xt0[0:1, :, 0:1])
        for c2 in range(0, CT, 2):
            nc.sync.dma_start(out=wk_sb[:, c2 : c2 + 2, :], in_=wk[:, c2 : c2 + 2, :])
            nc.scalar.dma_start(out=wv_sb[:, c2 : c2 + 2, :], in_=wv[:, c2 : c2 + 2, :])
        xt1 = xt_pool.tile([128, CT, 512], BF16, tag="xt", name="xt1")
        nc.sync.dma_start(out=xt1[0:1, 0:1, 0:1], in_=wk_sb[0:1, 0:1, 0:1])
        nc.sync.dma_start(out=xt1, in_=xT[1])
        tri_sb = consts.tile([128, 2, 128], BF16, tag="tri")
        nc.scalar.dma_start(out=tri_sb, in_=tri2)
        bq_sb = consts.tile([128, 4], F32, tag="bq")
        nc.scalar.dma_start(out=bq_sb, in_=bq.rearrange("(jt p) -> p jt", p=128))
        bk_sb = consts.tile([128, 4], F32, tag="bk")
        nc.scalar.dma_start(out=bk_sb, in_=bk.rearrange("(jt p) -> p jt", p=128))
        bv_sb = consts.tile([128, JC], F32, tag="bv")
        nc.scalar.dma_start(out=bv_sb, in_=bv.unsqueeze(0).to_broadcast([128, JC]))
        bp_sb = consts.tile([128, C], F32, tag="bp")
        wp_sb = w_pool.tile([128, 4, C], BF16, tag="wp")

        def load_wp():
            # deferred: wp/bp are first needed by oproj q0, well after
            # startup; the scheduler places these after attn q0's exps in
            # the scalar ring so they don't steal startup DMA bandwidth.
            nc.scalar.dma_start(out=wp_sb, in_=wp)
            nc.scalar.dma_start(out=bp_sb, in_=bp.unsqueeze(0).to_broadcast([128, C]))

        q_sb = [
            qk_pool.tile([128, T], BF16, tag=f"q{jt}", name=f"q{jt}")
            for jt in range(NP)
        ]
        k_sb = [
            qk_pool.tile([128, T], BF16, tag=f"k{jt}", name=f"k{jt}")
            for jt in range(NP)
        ]
        v_sb = [
            v_pool.tile([128, HL, 65], BF16, tag=f"v{tt}", name=f"v{tt}")
            for tt in range(T // 128)
        ]
        y_sb = [
            y_pool.tile([128, T], BF16, tag=f"y{jt}", name=f"y{jt}")
            for jt in range(NP)
        ]

        xts = {0: xt0, 1: xt1}

        def proj_units(c):
            # generator: yields after each psum-group (12 units per chunk)
            # prefetch chunk c+1 (its pool slot frees when chunk c-1 retires)
            if c + 1 <= 3 and c + 1 not in xts:
                nxt = xt_pool.tile([128, CT, 512], BF16, tag="xt", name=f"xt{c + 1}")
                nc.sync.dma_start(out=nxt, in_=xT[c + 1])
                xts[c + 1] = nxt
            xt_t = xts[c]
            for wsb, bsb, dst in ((wq_sb, bq_sb, q_sb), (wk_sb, bk_sb, k_sb)):
                for jt in range(NP):
                    ps = ppsum.tile([128, 512], F32, tag="pp")
                    for ct in range(CT):
                        nc.tensor.matmul(
                            ps,
                            lhsT=wsb[:, ct, ts(jt, 128)],
                            rhs=xt_t[:, ct, :],
                            start=(ct == 0),
                            stop=(ct == CT - 1),
                        )
                    nc.vector.tensor_scalar_add(
                        out=dst[jt][:, ts(c, 512)], in0=ps, scalar1=bsb[:, jt : jt + 1]
                    )
                    yield
            for sub in range(4):
                t128 = 4 * c + sub
                ps = ppsum.tile([128, 512], F32, tag="pp")
                for ct in range(CT):
                    nc.tensor.matmul(
                        ps,
                        lhsT=xt_t[:, ct, ts(sub, 128)],
                        rhs=wv_sb[:, ct, :],
                        start=(ct == 0),
                        stop=(ct == CT - 1),
                    )
                nc.vector.memset(v_sb[t128][:, :, 64:65], 1.0)
                nc.vector.tensor_tensor(
                    out=v_sb[t128][:, :, 0:64],
                    in0=ps.rearrange("p (h d) -> p h d", h=HL),
                    in1=bv_sb.rearrange("p (h d) -> p h d", h=HL),
                    op=ADD,
                )
                yield

        def attn_units(qc, prs=None):
            for pr in prs if prs is not None else range(NP):
                hA, hB = 2 * pr, 2 * pr + 1
                # one PV accumulation group per head, each owning a full PSUM
                # bank (start=True clears has_written for the WHOLE bank, so
                # two groups must never share one).
                pv = pv_pool.tile([128, 2, 512], F32, tag="pv", name=f"pv{qc}_{pr}")
                lastkt = 4 * qc + 3
                for kt in range(lastkt + 1):
                    off = max(0, 128 * (kt - 4 * qc))
                    sc = sc_pool.tile(
                        [128, 2, 512], F32, tag="sc", name=f"sc{qc}_{pr}_{kt}"
                    )
                    nc.tensor.matmul(
                        sc[:, 0, off:512],
                        lhsT=k_sb[pr][0:64, ts(kt, 128)],
                        rhs=q_sb[pr][0:64, 512 * qc + off : 512 * (qc + 1)],
                        start=True,
                        stop=True,
                        tile_position=(0, 0),
                    )
                    nc.tensor.matmul(
                        sc[:, 1, off:512],
                        lhsT=k_sb[pr][64:128, ts(kt, 128)],
                        rhs=q_sb[pr][64:128, 512 * qc + off : 512 * (qc + 1)],
                        start=True,
                        stop=True,
                        tile_position=(64, 0),
                    )
                    pt = pt_pool.tile(
                        [128, 2, 512], BF16, tag="pt", name=f"pt{qc}_{pr}_{kt}"
                    )
                    nc.scalar.activation(
                        out=pt[:, :, off:512], in_=sc[:, :, off:512], func=EXP, scale=0.125
                    )
                    if kt >= 4 * qc:
                        # causal mask of the diagonal 128x128 block, both heads
                        nc.vector.tensor_tensor(
                            out=pt[:, :, off : off + 128],
                            in0=pt[:, :, off : off + 128],
                            in1=tri_sb,
                            op=MULT,
                        )
                    for i, h in ((0, hA), (1, hB)):
                        nc.tensor.matmul(
                            pv[0:65, i, off:512],
                            lhsT=v_sb[kt][:, h, :],
                            rhs=pt[:, i, off:512],
                            start=(kt == 0),
                            stop=(kt == lastkt),
                        )
                    if kt != lastkt:
                        yield
                # evacuate PV psum in one DVE copy so the banks free fast;
                # the denominator DMA-bounce chain then runs off SBUF,
                # off the Tensor engine's critical path.
                pvs = pvs_pool.tile([65, 2, 512], F32, tag="pvs", name=f"pvs{qc}_{pr}")
                nc.vector.tensor_copy(out=pvs, in_=pv[0:65, :, :])
                dscr = d_pool.tile([1024], F32, tag="dscr", name=f"ds{qc}_{pr}")
                nc.sync.dma_start(out=dscr.unsqueeze(0), in_=pvs[64:65, :, :])
                rdb = rd_pool.tile([64, 1024], F32, tag="rdb", name=f"rb{qc}_{pr}")
                nc.sync.dma_start(
                    out=rdb, in_=dscr.unsqueeze(0).to_broadcast([64, 1024])
                )
                rcp = rd_pool.tile([64, 1024], F32, tag="rcp", name=f"rc{qc}_{pr}")
                nc.vector.reciprocal_approx_fast(out=rcp, in_=rdb)
                for i in range(2):
                    nc.vector.tensor_tensor(
                        out=y_sb[pr][64 * i : 64 * i + 64, ts(qc, 512)],
                        in0=pvs[0:64, i, :],
                        in1=rcp[:, ts(i, 512)],
                        op=MULT,
                    )
                yield

        def oproj_units(qc):
            for sub in range(4):
                t128 = 4 * qc + sub
                ot = o_pool.tile([128, C], F32, tag="ot", name=f"ot{t128}")
                for ch in range(2):
                    ps = ppsum.tile([128, 512], F32, tag="pp", name=f"op{t128}_{ch}")
                    for jt in range(NP):
                        nc.tensor.matmul(
                            ps,
                            lhsT=y_sb[jt][:, ts(t128, 128)],
                            rhs=wp_sb[:, jt, ts(ch, 512)],
                            start=(jt == 0),
                            stop=(jt == NP - 1),
                        )
                    nc.vector.tensor_tensor(
                        out=ot[:, ts(ch, 512)], in0=ps, in1=bp_sb[:, ts(ch, 512)], op=ADD
                    )
                nc.sync.dma_start(out=out[ts(t128, 128), :], in_=ot)
                yield

        op3_parts = {}

        def oproj3_pre():
            # jt 0-2 partial output projection for the last q chunk: runs
            # during the (ACT-bound) final attention pair, evacuated to SBUF
            # so only the 8 jt3 matmuls + adds remain after the last norm.
            for sub in range(4):
                t128 = 12 + sub
                for ch in range(2):
                    ps = ppsum.tile([128, 512], F32, tag="pp", name=f"o3a{sub}_{ch}")
                    for jt in range(3):
                        nc.tensor.matmul(
                            ps,
                            lhsT=y_sb[jt][:, ts(t128, 128)],
                            rhs=wp_sb[:, jt, ts(ch, 512)],
                            start=(jt == 0),
                            stop=(jt == 2),
                        )
                    part = part_pool.tile(
                        [128, 512], F32, tag="part", name=f"o3p{sub}_{ch}"
                    )
                    nc.vector.tensor_tensor(
                        out=part, in0=ps, in1=bp_sb[:, ts(ch, 512)], op=ADD
                    )
                    op3_parts[(sub, ch)] = part
                    yield

        def oproj3_post():
            for sub in range(4):
                t128 = 12 + sub
                ot = o_pool.tile([128, C], F32, tag="ot", name=f"ot{t128}")
                for ch in range(2):
                    ps = ppsum.tile([128, 512], F32, tag="pp", name=f"o3b{sub}_{ch}")
                    nc.tensor.matmul(
                        ps,
                        lhsT=y_sb[3][:, ts(t128, 128)],
                        rhs=wp_sb[:, 3, ts(ch, 512)],
                        start=True,
                        stop=True,
                    )
                    nc.vector.tensor_tensor(
                        out=ot[:, ts(ch, 512)],
                        in0=ps,
                        in1=op3_parts[(sub, ch)],
                        op=ADD,
                    )
                nc.sync.dma_start(out=out[ts(t128, 128), :], in_=ot)

        def chain(*gens):
            for g in gens:
                yield from g

        def drive(primary, filler, ratio):
            # emit filler units between primary units at `ratio` fillers per
            # primary: puts guaranteed-ready PE work right behind each
            # exp-gated attention unit in every engine queue.
            acc = 0.0
            filler_alive = True
            for _ in primary:
                acc += ratio
                while acc >= 1.0 and filler_alive:
                    try:
                        next(filler)
                    except StopIteration:
                        filler_alive = False
                    acc -= 1.0
            if filler_alive:
                for _ in filler:
                    pass

        # staircase: attn q-chunk c trails proj chunk c; oproj trails attn.
        # Attention units lead (their scores feed the ACT-bound softmax);
        # projection/oproj groups are interleaved as ready PE filler.
        for _ in proj_units(0):
            pass
        load_wp()
        drive(attn_units(0), proj_units(1), 12.0 / 16)
        drive(attn_units(1), chain(proj_units(2), oproj_units(0)), 16.0 / 32)
        drive(attn_units(2), chain(proj_units(3), oproj_units(1)), 16.0 / 48)
        drive(attn_units(3, range(3)), oproj_units(2), 4.0 / 48)
        drive(attn_units(3, (3,)), oproj3_pre(), 8.0 / 16)
        oproj3_post()


_CACHE = {}


def build_nc():
    if "nc" in _CACHE:
        return _CACHE["nc"]
    nc = bacc.Bacc(
        "TRN2",
        target_bir_lowering=False,
        debug=False,
        enable_asserts=False,
        num_devices=NCORES,
    )
    io = (
        nc.dram_tensor("xT", [NCH, 128, CT, 512], BF16, kind="ExternalInput").ap(),
        nc.dram_tensor("wq", [128, CT, JC], BF16, kind="ExternalInput").ap(),
        nc.dram_tensor("wk", [128, CT, JC], BF16, kind="ExternalInput").ap(),
        nc.dram_tensor("wv", [128, CT, JC], BF16, kind="ExternalInput").ap(),
        nc.dram_tensor("wp", [128, 4, C], BF16, kind="ExternalInput").ap(),
        nc.dram_tensor("bq", [JC], F32, kind="ExternalInput").ap(),
        nc.dram_tensor("bk", [JC], F32, kind="ExternalInput").ap(),
        nc.dram_tensor("bv", [JC], F32, kind="ExternalInput").ap(),
        nc.dram_tensor("bp", [C], F32, kind="ExternalInput").ap(),
        nc.dram_tensor("tri", [128, 2, 128], BF16, kind="ExternalInput").ap(),
        nc.dram_tensor("out", [T, C], F32, kind="ExternalOutput").ap(),
    )
    with tile.TileContext(nc) as tc:
        _trace(nc, tc, io)
    nc.compile()
    _CACHE["nc"] = nc
    return nc


def make_in_maps(x, w_attn, b_attn, w_proj, b_proj):
    import ml_dtypes

    BF = ml_dtypes.bfloat16

    def _swz_w(w):  # [C, JC] -> [128p, CT, JC], partition-major contiguous
        return np.ascontiguousarray(
            w.reshape(CT, 128, JC).transpose(1, 0, 2)
        ).astype(BF)

    tri = np.triu(np.ones((128, 128), dtype=np.float32))
    tri2 = np.ascontiguousarray(np.stack([tri, tri], axis=1)).astype(BF)
    zeros_c = np.zeros(C, dtype=np.float32)
    in_maps = []
    for core in range(NCORES):
        b, hh = core // 2, core % 2
        j0 = JC * hh
        # x[b].T is [C, T]; -> [chunk, 128p, ct, 512t] contiguous
        xTs = np.ascontiguousarray(
            x[b].T.reshape(CT, 128, NCH, 512).transpose(2, 1, 0, 3)
        ).astype(BF)
        in_maps.append(
            {
                "xT": xTs,
                "wq": _swz_w(w_attn[:, j0 : j0 + JC]),
                "wk": _swz_w(w_attn[:, C + j0 : C + j0 + JC]),
                "wv": _swz_w(w_attn[:, 2 * C + j0 : 2 * C + j0 + JC]),
                "wp": np.ascontiguousarray(
                    w_proj[j0 : j0 + JC, :].reshape(4, 128, C).transpose(1, 0, 2)
                ).astype(BF),
                "bq": np.ascontiguousarray(b_attn[j0 : j0 + JC]).astype(np.float32),
                "bk": np.ascontiguousarray(b_attn[C + j0 : C + j0 + JC]).astype(
                    np.float32
                ),
                "bv": np.ascontiguousarray(b_attn[2 * C + j0 : 2 * C + j0 + JC]).astype(
                    np.float32
                ),
                "bp": (b_proj.astype(np.float32) if hh == 0 else zeros_c),
                "tri": tri2,
            }
        )
    return in_maps


def gather(parts):
    out = np.empty((B, T, C), dtype=np.float32)
    for b in range(B):
        out[b] = parts[2 * b]["out"] + parts[2 * b + 1]["out"]
    return out


def kernel(x, w_attn, b_attn, w_proj, b_proj):
    x = np.asarray(x, dtype=np.float32)
    w_attn = np.asarray(w_attn, dtype=np.float32)
    b_attn = np.asarray(b_attn, dtype=np.float32)
    w_proj = np.asarray(w_proj, dtype=np.float32)
    b_proj = np.asarray(b_proj, dtype=np.float32)
    nc = build_nc()
    in_maps = make_in_maps(x, w_attn, b_attn, w_proj, b_proj)
    res = run_bass_kernel_spmd(nc, in_maps, core_ids=list(range(NCORES)))
    return gather(res.results)


if __name__ == "__main__":
    rng = np.random.default_rng(0)
    x = rng.standard_normal((B, T, C), dtype=np.float32)
    w_attn = rng.standard_normal((C, 3 * C), dtype=np.float32) / np.sqrt(C)
    b_attn = np.zeros(3 * C, np.float32)
    w_proj = rng.standard_normal((C, C), dtype=np.float32) / np.sqrt(C)
    b_proj = np.zeros(C, np.float32)
    out = kernel(x, w_attn, b_attn, w_proj, b_proj)
    print(out.shape, out.dtype, np.abs(out).mean())
